# revision 5
# baseline (speedup 1.0000x reference)
"""CMMD loss kernel for Trainium2 (Bass/Tile), 8-core SPMD — v6.

v6 over v5:
 - fp8(e4m3) X^T payload: the AllGather moves 1.06MB/rank instead of
   2.1MB (the bandwidth stats stay bf16 in the same buffer via bitcast
   views), and the Gram matmuls run on fp8 operands.  X ~ N(0,1) and the
   exp arguments are O(1), so fp8 quantization noise averages out far
   below the 2e-2 tolerance.
 - Engine rebalance in setup: big constant loads ride GpSimd, all AG-input
   writes ride the SP ring, ScalarE does only the 16 squares (its HWDGE
   ring only carries the other half of the loads), the exp-table warm
   dummy no longer blocks the ACT queue.
 - Diag block: gram+bias results spill PSUM->SBUF(bf16) during the AG;
   its exp/reduce groups defer into the task pipeline (one per gram point)
   so TensorE never stalls on the serial squaring chains at AG-end.
 - Esum chains split: squarings on VectorE, running sums on GpSimd --
   halves the chain latency and VectorE load.
"""

import dataclasses
from dataclasses import dataclass

import numpy as np
import ml_dtypes

import concourse.bass as bass
import concourse.bacc as bacc
import concourse.mybir as mybir
import concourse.tile as tile

F32 = mybir.dt.float32
F32R = mybir.dt.float32r
BF16 = mybir.dt.bfloat16
F8 = mybir.dt.float8e4
AX = mybir.AxisListType
ALU = mybir.AluOpType
ACTF = mybir.ActivationFunctionType


@dataclass(frozen=True)
class Cfg:
    n: int = 4096
    d: int = 2048
    cores: int = 8
    ncls: int = 8
    kernel_num: int = 5

    @property
    def rpc(self):
        return self.n // self.cores

    @property
    def ni(self):
        return self.rpc // 128

    @property
    def nk(self):
        return self.d // 128


CFG = Cfg()

SLOT_W = [512, 512, 512, 512, 256, 256]
SLOT_OFF = [0, 512, 1024, 1536, 2048, 2304]
VTW = 2560


def _build(cfg: Cfg):
    nc = bacc.Bacc(
        "TRN2",
        target_bir_lowering=False,
        debug=False,
        num_devices=cfg.cores,
    )
    NI, NK, NC = cfg.ni, cfg.nk, cfg.ncls
    D, RPC, N = cfg.d, cfg.rpc, cfg.n
    NL = cfg.kernel_num
    groups = [list(range(cfg.cores))]
    DRPC = D * RPC              # fp8 elements of X^T payload
    HK = DRPC // 2              # same region in bf16 elements
    TAIL = RPC + D + 1          # halfsq | colsum (k-major) | s1 (bf16)
    TAILP = 16384
    AGT = HK + TAILP            # per-rank payload in bf16 elements

    xsT = nc.dram_tensor("xsT", [D, RPC], F32, kind="ExternalInput").ap()
    vown = nc.dram_tensor("vown", [RPC, NC], BF16, kind="ExternalInput").ap()
    vt = nc.dram_tensor("vt", [NC, VTW], BF16, kind="ExternalInput").ap()
    cst = nc.dram_tensor("cst", [1, 16], F32, kind="ExternalInput").ap()
    cones = nc.dram_tensor("cones", [128, 1], F32, kind="ExternalInput").ap()
    cbcol = nc.dram_tensor("cbcol", [128, 1], BF16, kind="ExternalInput").ap()
    crow = nc.dram_tensor("crow", [1, 128], F32, kind="ExternalInput").ap()
    cnrow = nc.dram_tensor("cnrow", [1, 128], F32, kind="ExternalInput").ap()
    partial = nc.dram_tensor("partial", [1, 1], F32, kind="ExternalOutput").ap()

    with tile.TileContext(nc) as tc:
        with (
            tc.tile_pool(name="dram", bufs=1, space="DRAM") as dram,
            tc.tile_pool(name="pers", bufs=1) as pers,
        ):
            agi = dram.tile([AGT], BF16)
            ag_all = dram.tile([cfg.cores * AGT], BF16, addr_space="Shared")
            hsd = dram.tile([RPC], F32)

            ones_col = pers.tile([128, 1], F32)
            bones_col = pers.tile([128, 1], BF16)
            ones_row = pers.tile([1, 128], F32)
            negs_row = pers.tile([1, 128], F32)
            negs_rowr = pers.tile([1, 128], F32R)
            cst_sb = pers.tile([1, 16], F32)
            vown_sb = pers.tile([128, NI, NC], BF16)
            vt_sb = pers.tile([NC, VTW], BF16)
            hsrow = pers.tile([1, RPC], F32)
            halfsq = pers.tile([128, NI], F32)
            colsum = pers.tile([128, NK], F32)
            ag_sb = pers.tile([cfg.cores, TAIL], BF16)
            sc = pers.tile([128, 2 * NL], F32)
            biases = pers.tile([128, NL * NI], F32)
            loss_cols = pers.tile([NC, 8], F32)
            lred = pers.tile([NC, 1], F32)
            out_sb = pers.tile([1, 1], F32)
            s2v = pers.tile([1, 4], F32)
            junk_e = pers.tile([1, 16], BF16)
            gdiag = [pers.tile([128, RPC], BF16, name=f"gd{i}") for i in range(NI)]
            xto = pers.tile([128, NK, RPC], F8)

            # tiny consts on the SP ring, big tables on gpsimd
            nc.sync.dma_start(cst_sb[:], cst)
            nc.sync.dma_start(ones_col[:], cones)
            nc.sync.dma_start(bones_col[:], cbcol)
            nc.sync.dma_start(ones_row[:], crow)
            nc.sync.dma_start(negs_row[:], cnrow)
            nc.vector.tensor_copy(negs_rowr[:], negs_row[:])
            nc.gpsimd.dma_start(vown_sb[:], vown.rearrange("(i p) c -> p i c", p=128))
            nc.gpsimd.dma_start(vt_sb[:], vt)

            hw_rings = [nc.sync, nc.scalar]

            with (
                tc.tile_pool(name="setup", bufs=1) as setup,
                tc.tile_pool(name="work", bufs=1) as work,
                tc.tile_pool(name="mpsum", bufs=1, space="PSUM") as mpsum,
            ):
                # ---- all 16 chunk loads first, split across both rings
                xq = []
                for k in range(NK):
                    t = setup.tile([128, RPC], F32, tag="xq", bufs=NK)
                    hw_rings[k % 2].dma_start(t[:], xsT[128 * k : 128 * (k + 1), :])
                    xq.append(t)
                # warm the exp table (queued on ACT after its load issues)
                nc.scalar.activation(junk_e[:], cst_sb[:], ACTF.Exp, scale=0.0)

                # ---- per chunk: square (ACT), rownorm-MM (PE), cast+colsum
                # (DVE), payload write (SP ring)
                ps_hs_t = mpsum.tile([128, 512], F32, tag="g", bufs=6, name="pshs")
                ps_hs = ps_hs_t[0:1, 0:RPC]
                for k in range(NK):
                    sq = setup.tile([128, RPC], BF16, tag="sq", bufs=3)
                    nc.scalar.activation(
                        sq[:], xq[k][:], ACTF.Square, scale=float(np.sqrt(0.5))
                    )
                    nc.tensor.matmul(
                        ps_hs,
                        lhsT=bones_col[:],
                        rhs=sq[:],
                        start=(k == 0),
                        stop=(k == NK - 1),
                    )
                    nc.vector.tensor_copy(xto[:, k, :], xq[k][:])
                    nc.vector.tensor_reduce(
                        colsum[:, k : k + 1], xq[k][:], axis=AX.X, op=ALU.add
                    )
                    hw_rings[k % 2].dma_start(
                        agi[k * 64 * RPC : (k + 1) * 64 * RPC]
                        .bitcast(F8)
                        .rearrange("(p j) -> p j", p=128),
                        xto[:, k, :],
                    )

                nc.vector.tensor_copy(hsrow[:], ps_hs)
                s1p = setup.tile([1, 1], F32, tag="tiny", bufs=8)
                nc.vector.tensor_reduce(s1p[:], hsrow[:], axis=AX.X, op=ALU.add)

                # stats ride in the AG payload as bf16 (cast during DMA)
                nc.gpsimd.dma_start(
                    agi[HK : HK + RPC].rearrange("(o c) -> o c", o=1), hsrow[:]
                )
                # p-major layout: 128 contiguous 32B runs instead of 2048
                # 2-byte descriptors (whose drain gated the AG doorbell);
                # downstream only needs sum-of-squares, order-agnostic
                nc.gpsimd.dma_start(
                    agi[HK + RPC : HK + RPC + D].rearrange("(p k) -> p k", p=128),
                    colsum[:],
                )
                nc.gpsimd.dma_start(
                    agi[HK + RPC + D : HK + TAIL].rearrange("(o c) -> o c", o=1),
                    s1p[:],
                )

                ag_big = nc.gpsimd.collective_compute(
                    "AllGather",
                    ALU.bypass,
                    replica_groups=groups,
                    ins=[agi[:].opt()],
                    outs=[ag_all[:].opt()],
                )

                # f32 bounce for partition-major own-row norms (during the AG)
                nc.sync.dma_start(hsd[:].rearrange("(o c) -> o c", o=1), hsrow[:])
                nc.scalar.dma_start(
                    halfsq[:], hsd[:].rearrange("(t p) -> p t", p=128)
                )

                # ---- diag gram during the AG; spill to SBUF to free PSUM
                hsj_d16 = work.tile([1, 512], BF16, tag="hsj16", bufs=8)
                nc.sync.dma_start(
                    hsj_d16[:], agi[HK : HK + RPC].rearrange("(o c) -> o c", o=1)
                )
                hsj_d = work.tile([1, 512], F32R, tag="hsj", bufs=8)
                nc.vector.tensor_copy(hsj_d[:], hsj_d16[:])
                for i in range(NI):
                    g = mpsum.tile([128, 512], F32, tag="g", bufs=6, name=f"gd{i}")
                    for k2 in range(NK // 2):
                        nc.tensor.matmul(
                            g,
                            lhsT=xto[:, 2 * k2 : 2 * k2 + 2, 128 * i : 128 * (i + 1)],
                            rhs=xto[:, 2 * k2 : 2 * k2 + 2, :],
                            start=(k2 == 0),
                            stop=False,
                            perf_mode=mybir.MatmulPerfMode.DoubleRow,
                        )
                    nc.tensor.matmul(
                        g, lhsT=negs_rowr[:], rhs=hsj_d[:], start=False, stop=True
                    )
                    nc.vector.tensor_copy(gdiag[i][:], g[:])

                # ---- dynamic per-core offsets
                pid_s = nc.scalar.partition_id()
                pid_y = nc.sync.partition_id()
                pid_g = nc.gpsimd.partition_id()

                def offs(pid, unit, base):
                    # offsets in `unit`-per-bf16 elements with region base
                    sub1 = ((pid & 4) << 6) * 1
                    sub2 = (((pid + 4) & 4) << 6) * 1
                    xo = {}
                    for s in (1, 2, 3):
                        xo[s] = ((pid + s) & 7) * (AGT * unit) + base
                    r4 = (pid + 4) & 7
                    xo[4] = r4 * (AGT * unit) + base + sub1
                    xo[5] = r4 * (AGT * unit) + base + sub2
                    return xo

                xoff_s = offs(pid_s, 2, 0)
                xoff_y = offs(pid_y, 2, 0)
                hoff = offs(pid_g, 1, HK)

                def xt_dyn(w, off_rv, k0, kcnt):
                    ap0 = (
                        ag_all[k0 * 64 * RPC : (k0 + kcnt) * 64 * RPC]
                        .bitcast(F8)
                        .rearrange("(k p j) -> p k j", p=128, j=RPC)[:, :, 0:w]
                    )
                    return dataclasses.replace(
                        ap0,
                        offset=off_rv + ap0.offset,
                        dep_tracking_offset=ap0.offset,
                    )

                def hs_dyn(w, off_rv):
                    ap0 = ag_all[0:w].rearrange("(o c) -> o c", o=1)
                    return dataclasses.replace(
                        ap0,
                        offset=off_rv + ap0.offset,
                        dep_tracking_offset=ap0.offset,
                    )

                tasks = [
                    (0, 512, [0, 1, 2, 3]),
                    (1, 512, [0, 1, 2, 3]),
                    (4, 256, [0, 1]),
                    (2, 512, [0, 1, 2, 3]),
                    (5, 256, [2, 3]),
                    (3, 512, [0, 1, 2, 3]),
                ]

                xtj = {}
                for s, w, _ in tasks[1:]:
                    tag = "xtj" if w == 512 else "xtj2"
                    bufs = 3 if w == 512 else 2
                    tl = work.tile([128, NK, w], F8, tag=tag, bufs=bufs)
                    if s == 1:
                        q = NK // 4
                        for j in range(4):
                            ring = hw_rings[j % 2]
                            xo = xoff_y if ring is nc.sync else xoff_s
                            ring.dma_start(
                                tl[:, j * q : (j + 1) * q, :],
                                xt_dyn(w, xo[s], j * q, q),
                            )
                    else:
                        half = NK // 2
                        nc.sync.dma_start(
                            tl[:, 0:half, :], xt_dyn(w, xoff_y[s], 0, half)
                        )
                        nc.scalar.dma_start(
                            tl[:, half:NK, :], xt_dyn(w, xoff_s[s], half, half)
                        )
                    xtj[s] = tl

                hsjr = {0: hsj_d}
                for s, w, _ in tasks[1:]:
                    h16 = work.tile([1, 512], BF16, tag="hsj16", bufs=8)
                    nc.gpsimd.dma_start(h16[:, 0:w], hs_dyn(w, hoff[s]))
                    hr = work.tile([1, 512], F32R, tag="hsj", bufs=8)
                    nc.vector.tensor_copy(hr[:, 0:w], h16[:, 0:w])
                    hsjr[s] = hr

                # ---- bandwidth from the gathered tails (after AG)
                ap_tail = ag_all[0 : cfg.cores * AGT].rearrange(
                    "(r c) -> r c", c=AGT
                )[:, HK : HK + TAIL]
                nc.scalar.dma_start(ag_sb[:], ap_tail)
                psum_s_t = mpsum.tile([NC, 512], F32, tag="R", bufs=2, name="ps1")
                psum_s = psum_s_t[0:1, 0:1]
                nc.tensor.matmul(
                    psum_s,
                    lhsT=bones_col[0 : cfg.cores, :],
                    rhs=ag_sb[:, RPC + D : RPC + D + 1],
                    start=True,
                    stop=True,
                )
                s1 = setup.tile([1, 1], F32, tag="tiny", bufs=8)
                nc.vector.tensor_copy(s1[:], psum_s)
                for ch in range(4):
                    pcg_t = mpsum.tile([128, 512], F32, tag="g", bufs=6, name=f"pcg{ch}")
                    pcg = pcg_t[0:1, 0:512]
                    nc.tensor.matmul(
                        pcg,
                        lhsT=bones_col[0 : cfg.cores, :],
                        rhs=ag_sb[:, RPC + 512 * ch : RPC + 512 * (ch + 1)],
                        start=True,
                        stop=True,
                    )
                    junk_cg = setup.tile([1, 512], BF16, tag="junkcg", bufs=2)
                    nc.scalar.activation(
                        junk_cg[:], pcg, ACTF.Square, accum_out=s2v[:, ch : ch + 1]
                    )
                s2 = setup.tile([1, 1], F32, tag="tiny", bufs=8)
                nc.vector.tensor_reduce(s2[:], s2v[:], axis=AX.X, op=ALU.add)
                t1 = setup.tile([1, 1], F32, tag="tiny", bufs=8)
                t2 = setup.tile([1, 1], F32, tag="tiny", bufs=8)
                bw0 = setup.tile([1, 1], F32, tag="tiny", bufs=8)
                inv0 = setup.tile([1, 1], F32, tag="tiny", bufs=8)
                nc.vector.tensor_scalar_mul(t1[:], s1[:], 1.0 / (N - 1))
                nc.vector.tensor_scalar_mul(t2[:], s2[:], -1.0 / (2.0 * N * (N - 1)))
                nc.vector.tensor_tensor(bw0[:], t1[:], t2[:], op=ALU.add)
                nc.vector.reciprocal(inv0[:], bw0[:])
                sc10 = setup.tile([1, 2 * NL], F32, tag="sc10", bufs=1)
                nc.vector.tensor_scalar_mul(sc10[:], cst_sb[:, 0 : 2 * NL], inv0[:])
                psum_b_t = mpsum.tile([128, 512], F32, tag="g", bufs=6, name="pb")
                psum_b = psum_b_t[:, 0 : 2 * NL]
                nc.tensor.matmul(
                    psum_b, lhsT=ones_row[:], rhs=sc10[:], start=True, stop=True
                )
                nc.vector.tensor_copy(sc[:], psum_b)
                for l in range(NL):
                    nc.vector.tensor_scalar_mul(
                        biases[:, NI * l : NI * (l + 1)],
                        halfsq[:],
                        sc[:, NL + l : NL + l + 1],
                    )

                # ---- main loop: deferred exp/Esum/reduce groups
                pend = []

                def flush_pend():
                    if not pend:
                        return
                    s, w, i, g, psum_R, first, last = pend.pop(0)
                    # levels 4,3,2 straight from ACT; 1,0 by squaring E2 on DVE
                    Es = []
                    for l in (NL - 1, NL - 2, NL - 3):
                        E = work.tile([128, 512], BF16, tag="E", bufs=24)
                        nc.scalar.activation(
                            E[:, 0:w],
                            g[:, 0:w],
                            ACTF.Exp,
                            bias=biases[:, NI * l + i : NI * l + i + 1],
                            scale=sc[:, l : l + 1],
                        )
                        Es.append(E)
                    S = Es[0]
                    for E_next in Es[1:]:
                        S2 = work.tile([128, 512], BF16, tag="E", bufs=24)
                        nc.vector.tensor_tensor(
                            S2[:, 0:w], S[:, 0:w], E_next[:, 0:w], op=ALU.add
                        )
                        S = S2
                    E = Es[-1]
                    for step in range(2):
                        E2 = work.tile([128, 512], BF16, tag="E", bufs=24)
                        nc.vector.tensor_tensor(
                            E2[:, 0:w], E[:, 0:w], E[:, 0:w], op=ALU.mult
                        )
                        S2 = work.tile([128, 512], BF16, tag="E", bufs=24)
                        nc.vector.tensor_tensor(
                            S2[:, 0:w], S[:, 0:w], E2[:, 0:w], op=ALU.add
                        )
                        E = E2
                        S = S2
                    nc.tensor.matmul(
                        psum_R[:, 0:w],
                        lhsT=vown_sb[:, i, :],
                        rhs=S[:, 0:w],
                        start=first,
                        stop=last,
                    )
                    if last:
                        scr = work.tile([NC, 512], F32, tag="scr", bufs=2)
                        nc.vector.tensor_tensor(
                            scr[:, 0:w],
                            psum_R[:, 0:w],
                            vt_sb[:, SLOT_OFF[s] : SLOT_OFF[s] + w],
                            op=ALU.mult,
                        )
                        nc.vector.tensor_reduce(
                            loss_cols[:, s : s + 1], scr[:, 0:w], axis=AX.X, op=ALU.add
                        )

                for s, w, ilist in tasks:
                    psum_R = mpsum.tile([NC, 512], F32, tag="R", bufs=2, name=f"R{s}")
                    for idx, i in enumerate(ilist):
                        if s == 0:
                            g = gdiag[i]  # spilled SBUF copy; no gram here
                        else:
                            g = mpsum.tile(
                                [128, 512], F32, tag="g", bufs=6, name=f"g{s}_{i}"
                            )
                            for k2 in range(NK // 2):
                                nc.tensor.matmul(
                                    g[:, 0:w],
                                    lhsT=xto[
                                        :, 2 * k2 : 2 * k2 + 2, 128 * i : 128 * (i + 1)
                                    ],
                                    rhs=xtj[s][:, 2 * k2 : 2 * k2 + 2, :],
                                    start=(k2 == 0),
                                    stop=False,
                                    perf_mode=mybir.MatmulPerfMode.DoubleRow,
                                )
                            nc.tensor.matmul(
                                g[:, 0:w],
                                lhsT=negs_rowr[:],
                                rhs=hsjr[s][:, 0:w],
                                start=False,
                                stop=True,
                            )
                            if len(pend) > 3:
                                flush_pend()
                            if len(pend) > 4:
                                flush_pend()
                        pend.append(
                            (s, w, i, g, psum_R, idx == 0, idx == len(ilist) - 1)
                        )
                while pend:
                    flush_pend()

                nc.vector.tensor_reduce(
                    lred[:], loss_cols[:, 0:6], axis=AX.X, op=ALU.add
                )
                psum_f = mpsum.tile([1, 1], F32, tag="R", bufs=2)
                nc.tensor.matmul(
                    psum_f[:],
                    lhsT=lred[:],
                    rhs=ones_col[0:NC, :],
                    start=True,
                    stop=True,
                )
                nc.vector.tensor_copy(out_sb[:], psum_f[:])
                nc.sync.dma_start(partial, out_sb[:])

    nc.compile()
    return nc


def host_prep(cfg: Cfg, source, target, s_label, t_label):
    """Slice/encode inputs into per-core in_maps (layout only, no math on X)."""
    X = np.concatenate(
        [np.asarray(source, np.float32), np.asarray(target, np.float32)], 0
    )
    bs = np.asarray(source).shape[0]
    lab = np.concatenate(
        [np.asarray(s_label).astype(np.int64), np.asarray(t_label).astype(np.int64)]
    )
    sign = np.ones(cfg.n, np.float32)
    sign[bs:] = -1.0
    V = np.zeros((cfg.n, cfg.ncls), np.float32)
    V[np.arange(cfg.n), lab] = sign
    Vb = V.astype(ml_dtypes.bfloat16)
    VT = np.ascontiguousarray(V.T)
    VT2 = 2.0 * VT

    NL = cfg.kernel_num
    cst = np.zeros((1, 16), np.float32)
    for l in range(NL):
        cst[0, l] = 2.0 * 2.0 ** (-l)
        cst[0, NL + l] = -2.0 * 2.0 ** (-l)
    cones = np.ones((128, 1), np.float32)
    cbcol = np.ones((128, 1), ml_dtypes.bfloat16)
    crow = np.ones((1, 128), np.float32)
    cnrow = -np.ones((1, 128), np.float32)

    in_maps = []
    for c in range(cfg.cores):
        r0, r1 = c * cfg.rpc, (c + 1) * cfg.rpc
        sub1 = 256 if c >= 4 else 0
        sub2 = 256 - sub1
        r4 = (c + 4) % 8
        slots = [VT[:, r0:r1]]
        for s in (1, 2, 3):
            rs = (c + s) % 8
            slots.append(VT2[:, 512 * rs : 512 * (rs + 1)])
        slots.append(VT2[:, 512 * r4 + sub1 : 512 * r4 + sub1 + 256])
        slots.append(VT2[:, 512 * r4 + sub2 : 512 * r4 + sub2 + 256])
        vt_c = np.concatenate(slots, axis=1).astype(ml_dtypes.bfloat16)
        in_maps.append(
            {
                "xsT": np.ascontiguousarray(X[r0:r1].T),
                "vown": np.ascontiguousarray(Vb[r0:r1]),
                "vt": np.ascontiguousarray(vt_c),
                "cst": cst,
                "cones": cones,
                "cbcol": cbcol,
                "crow": crow,
                "cnrow": cnrow,
            }
        )
    return in_maps


_NC_CACHE = {}


def _get_nc(cfg: Cfg):
    key = cfg
    if key not in _NC_CACHE:
        _NC_CACHE[key] = _build(cfg)
    return _NC_CACHE[key]


def run(inputs: dict, cfg: Cfg = CFG, trace: bool = False):
    from concourse.bass_utils import run_bass_kernel_spmd

    nc = _get_nc(cfg)
    in_maps = host_prep(
        cfg,
        inputs["source"],
        inputs["target"],
        inputs["s_label"],
        inputs["t_label"],
    )
    res = run_bass_kernel_spmd(
        nc, in_maps, core_ids=list(range(cfg.cores)), trace=trace
    )
    bs = np.asarray(inputs["source"]).shape[0]
    total = sum(float(r["partial"][0, 0]) for r in res.results)
    loss = np.float32(total / float(bs) ** 2)
    return np.asarray(loss, dtype=np.float32), res


def kernel(**inputs) -> np.ndarray:
    out, _ = run(inputs)
    return out


# revision 6
# speedup vs baseline: 1.0844x; 1.0844x over previous
"""CMMD loss kernel for Trainium2 (Bass/Tile), 8-core SPMD.

Math (reference semantics):
  X = concat(source, target)            [N=4096, D=2048]
  L2[i,j] = ||X_i - X_j||^2  (Gram trick);  bw = sum(L2)/(N^2-N)/4
  K = sum_l exp(-L2 / (bw 2^l));  loss = (1/Bs^2) sum_ij w_i.w_j K_ij

Design (one SPMD program on 8 cores):
 - Host hands each core its row shard pre-transposed (f32, layout only).
   Setup: 16 chunk loads; ScalarE squares -> row norms via ones-matmul;
   VectorE casts to fp8(e4m3) + column sums; one merged AllGather carries
   the fp8 X^T payload plus bf16 norm/colsum stats (padded to a 32KiB
   multiple -- ragged payloads measurably degrade RDH bandwidth).
 - Symmetry: each core computes its diagonal 512x512 block (x1) plus 3.5
   off-diagonal 512-blocks (x2), halving the Gram/exp work.  The per-core
   block rotation uses dynamic DMA offsets from partition_id registers;
   V^T slot weights are pre-rotated per core on the host.
 - The diagonal block is computed during the AllGather and spilled to
   SBUF; rhs blocks prefetch on both HWDGE rings the moment the AG lands.
 - Gram matmuls run fp8 DoubleRow (K=256 per pass, half the matmuls).
 - Kernel levels 4,3,2 come straight from ScalarE exp (scale/bias fold
   the bandwidth and row norms); levels 1,0 by squaring on VectorE; the
   level sum feeds ONE V^T @ S matmul per row tile.  Exp/reduce groups
   defer ~3 gram-points behind TensorE so it never waits on the chains.
 - Per-core partial scalars are summed on the host (labels-only math).
"""

import dataclasses
from dataclasses import dataclass

import numpy as np
import ml_dtypes

import concourse.bass as bass
import concourse.bacc as bacc
import concourse.mybir as mybir
import concourse.tile as tile

F32 = mybir.dt.float32
F32R = mybir.dt.float32r
BF16 = mybir.dt.bfloat16
F8 = mybir.dt.float8e4
AX = mybir.AxisListType
ALU = mybir.AluOpType
ACTF = mybir.ActivationFunctionType


@dataclass(frozen=True)
class Cfg:
    n: int = 4096
    d: int = 2048
    cores: int = 8
    ncls: int = 8
    kernel_num: int = 5

    @property
    def rpc(self):
        return self.n // self.cores

    @property
    def ni(self):
        return self.rpc // 128

    @property
    def nk(self):
        return self.d // 128


CFG = Cfg()

SLOT_W = [512, 512, 512, 512, 256, 256]
SLOT_OFF = [0, 512, 1024, 1536, 2048, 2304]
VTW = 2560


def _build(cfg: Cfg):
    nc = bacc.Bacc(
        "TRN2",
        target_bir_lowering=False,
        debug=False,
        num_devices=cfg.cores,
    )
    NI, NK, NC = cfg.ni, cfg.nk, cfg.ncls
    D, RPC, N = cfg.d, cfg.rpc, cfg.n
    NL = cfg.kernel_num
    groups = [list(range(cfg.cores))]
    DRPC = D * RPC              # fp8 elements of X^T payload
    HK = DRPC // 2              # same region in bf16 elements
    TAIL = RPC + D + 1          # halfsq | colsum (k-major) | s1 (bf16)
    TAILP = 16384
    AGT = HK + TAILP            # per-rank payload in bf16 elements

    xsT = nc.dram_tensor("xsT", [D, RPC], F32, kind="ExternalInput").ap()
    vown = nc.dram_tensor("vown", [RPC, NC], BF16, kind="ExternalInput").ap()
    vt = nc.dram_tensor("vt", [NC, VTW], BF16, kind="ExternalInput").ap()
    cst = nc.dram_tensor("cst", [1, 16], F32, kind="ExternalInput").ap()
    cones = nc.dram_tensor("cones", [128, 1], F32, kind="ExternalInput").ap()
    cbcol = nc.dram_tensor("cbcol", [128, 1], BF16, kind="ExternalInput").ap()
    crow = nc.dram_tensor("crow", [1, 128], F32, kind="ExternalInput").ap()
    cnrow = nc.dram_tensor("cnrow", [1, 128], F32, kind="ExternalInput").ap()
    partial = nc.dram_tensor("partial", [1, 1], F32, kind="ExternalOutput").ap()

    with tile.TileContext(nc) as tc:
        with (
            tc.tile_pool(name="dram", bufs=1, space="DRAM") as dram,
            tc.tile_pool(name="pers", bufs=1) as pers,
        ):
            agi = dram.tile([AGT], BF16)
            ag_all = dram.tile([cfg.cores * AGT], BF16, addr_space="Shared")
            hsd = dram.tile([RPC], F32)

            ones_col = pers.tile([128, 1], F32)
            bones_col = pers.tile([128, 1], BF16)
            ones_row = pers.tile([1, 128], F32)
            negs_row = pers.tile([1, 128], F32)
            negs_rowr = pers.tile([1, 128], F32R)
            cst_sb = pers.tile([1, 16], F32)
            vown_sb = pers.tile([128, NI, NC], BF16)
            vt_sb = pers.tile([NC, VTW], BF16)
            hsrow = pers.tile([1, RPC], F32)
            halfsq = pers.tile([128, NI], F32)
            colsum = pers.tile([128, NK], F32)
            ag_sb = pers.tile([cfg.cores, TAIL], BF16)
            sc = pers.tile([128, 2 * NL], F32)
            biases = pers.tile([128, NL * NI], F32)
            loss_cols = pers.tile([NC, 8], F32)
            lred = pers.tile([NC, 1], F32)
            out_sb = pers.tile([1, 1], F32)
            s2v = pers.tile([1, 4], F32)
            junk_e = pers.tile([1, 16], BF16)
            gdiag = [pers.tile([128, RPC], BF16, name=f"gd{i}") for i in range(NI)]
            xto = pers.tile([128, NK, RPC], F8)

            # tiny consts on the SP ring, big tables on gpsimd
            nc.sync.dma_start(cst_sb[:], cst)
            nc.sync.dma_start(ones_col[:], cones)
            nc.sync.dma_start(bones_col[:], cbcol)
            nc.sync.dma_start(ones_row[:], crow)
            nc.sync.dma_start(negs_row[:], cnrow)
            nc.vector.tensor_copy(negs_rowr[:], negs_row[:])
            nc.gpsimd.dma_start(vown_sb[:], vown.rearrange("(i p) c -> p i c", p=128))
            nc.gpsimd.dma_start(vt_sb[:], vt)

            hw_rings = [nc.sync, nc.scalar]

            with (
                tc.tile_pool(name="setup", bufs=1) as setup,
                tc.tile_pool(name="work", bufs=1) as work,
                tc.tile_pool(name="mpsum", bufs=1, space="PSUM") as mpsum,
            ):
                # ---- all 16 chunk loads first, split across both rings
                xq = []
                for k in range(NK):
                    t = setup.tile([128, RPC], F32, tag="xq", bufs=NK)
                    hw_rings[k % 2].dma_start(t[:], xsT[128 * k : 128 * (k + 1), :])
                    xq.append(t)
                # warm the exp table (queued on ACT after its load issues)
                nc.scalar.activation(junk_e[:], cst_sb[:], ACTF.Exp, scale=0.0)

                # ---- per chunk: square (ACT), rownorm-MM (PE), cast+colsum
                # (DVE), payload write (SP ring)
                ps_hs_t = mpsum.tile([128, 512], F32, tag="g", bufs=6, name="pshs")
                ps_hs = ps_hs_t[0:1, 0:RPC]
                for k in range(NK):
                    sq = setup.tile([128, RPC], BF16, tag="sq", bufs=3)
                    nc.scalar.activation(
                        sq[:], xq[k][:], ACTF.Square, scale=float(np.sqrt(0.5))
                    )
                    nc.tensor.matmul(
                        ps_hs,
                        lhsT=bones_col[:],
                        rhs=sq[:],
                        start=(k == 0),
                        stop=(k == NK - 1),
                    )
                    nc.vector.tensor_copy(xto[:, k, :], xq[k][:])
                    nc.vector.tensor_reduce(
                        colsum[:, k : k + 1], xq[k][:], axis=AX.X, op=ALU.add
                    )
                    hw_rings[k % 2].dma_start(
                        agi[k * 64 * RPC : (k + 1) * 64 * RPC]
                        .bitcast(F8)
                        .rearrange("(p j) -> p j", p=128),
                        xto[:, k, :],
                    )

                nc.vector.tensor_copy(hsrow[:], ps_hs)
                s1p = setup.tile([1, 1], F32, tag="tiny", bufs=8)
                nc.vector.tensor_reduce(s1p[:], hsrow[:], axis=AX.X, op=ALU.add)

                # stats ride in the AG payload as bf16 (cast during DMA)
                nc.gpsimd.dma_start(
                    agi[HK : HK + RPC].rearrange("(o c) -> o c", o=1), hsrow[:]
                )
                # p-major layout: 128 contiguous 32B runs instead of 2048
                # 2-byte descriptors (whose drain gated the AG doorbell);
                # downstream only needs sum-of-squares, order-agnostic
                nc.gpsimd.dma_start(
                    agi[HK + RPC : HK + RPC + D].rearrange("(p k) -> p k", p=128),
                    colsum[:],
                )
                nc.gpsimd.dma_start(
                    agi[HK + RPC + D : HK + TAIL].rearrange("(o c) -> o c", o=1),
                    s1p[:],
                )

                ag_big = nc.gpsimd.collective_compute(
                    "AllGather",
                    ALU.bypass,
                    replica_groups=groups,
                    ins=[agi[:].opt()],
                    outs=[ag_all[:].opt()],
                )

                # f32 bounce for partition-major own-row norms (during the AG)
                nc.sync.dma_start(hsd[:].rearrange("(o c) -> o c", o=1), hsrow[:])
                nc.scalar.dma_start(
                    halfsq[:], hsd[:].rearrange("(t p) -> p t", p=128)
                )

                # ---- diag gram during the AG; spill to SBUF to free PSUM
                hsj_d16 = work.tile([1, 512], BF16, tag="hsj16", bufs=8)
                nc.sync.dma_start(
                    hsj_d16[:], agi[HK : HK + RPC].rearrange("(o c) -> o c", o=1)
                )
                hsj_d = work.tile([1, 512], F32R, tag="hsj", bufs=8)
                nc.vector.tensor_copy(hsj_d[:], hsj_d16[:])
                for i in range(NI):
                    g = mpsum.tile([128, 512], F32, tag="g", bufs=6, name=f"gd{i}")
                    for k2 in range(NK // 2):
                        nc.tensor.matmul(
                            g,
                            lhsT=xto[:, 2 * k2 : 2 * k2 + 2, 128 * i : 128 * (i + 1)],
                            rhs=xto[:, 2 * k2 : 2 * k2 + 2, :],
                            start=(k2 == 0),
                            stop=False,
                            perf_mode=mybir.MatmulPerfMode.DoubleRow,
                        )
                    nc.tensor.matmul(
                        g, lhsT=negs_rowr[:], rhs=hsj_d[:], start=False, stop=True
                    )
                    nc.vector.tensor_copy(gdiag[i][:], g[:])

                # ---- dynamic per-core offsets
                pid_s = nc.scalar.partition_id()
                pid_y = nc.sync.partition_id()
                pid_g = nc.gpsimd.partition_id()

                def offs(pid, unit, base):
                    # offsets in `unit`-per-bf16 elements with region base
                    sub1 = ((pid & 4) << 6) * 1
                    sub2 = (((pid + 4) & 4) << 6) * 1
                    xo = {}
                    for s in (1, 2, 3):
                        xo[s] = ((pid + s) & 7) * (AGT * unit) + base
                    r4 = (pid + 4) & 7
                    xo[4] = r4 * (AGT * unit) + base + sub1
                    xo[5] = r4 * (AGT * unit) + base + sub2
                    return xo

                xoff_s = offs(pid_s, 2, 0)
                xoff_y = offs(pid_y, 2, 0)
                hoff = offs(pid_g, 1, HK)

                def xt_dyn(w, off_rv, k0, kcnt):
                    ap0 = (
                        ag_all[k0 * 64 * RPC : (k0 + kcnt) * 64 * RPC]
                        .bitcast(F8)
                        .rearrange("(k p j) -> p k j", p=128, j=RPC)[:, :, 0:w]
                    )
                    return dataclasses.replace(
                        ap0,
                        offset=off_rv + ap0.offset,
                        dep_tracking_offset=ap0.offset,
                    )

                def hs_dyn(w, off_rv):
                    ap0 = ag_all[0:w].rearrange("(o c) -> o c", o=1)
                    return dataclasses.replace(
                        ap0,
                        offset=off_rv + ap0.offset,
                        dep_tracking_offset=ap0.offset,
                    )

                tasks = [
                    (0, 512, [0, 1, 2, 3]),
                    (1, 512, [0, 1, 2, 3]),
                    (4, 256, [0, 1]),
                    (2, 512, [0, 1, 2, 3]),
                    (5, 256, [2, 3]),
                    (3, 512, [0, 1, 2, 3]),
                ]

                xtj = {}
                for s, w, _ in tasks[1:]:
                    tag = "xtj" if w == 512 else "xtj2"
                    bufs = 3 if w == 512 else 2
                    tl = work.tile([128, NK, w], F8, tag=tag, bufs=bufs)
                    if s == 1:
                        q = NK // 4
                        for j in range(4):
                            ring = hw_rings[j % 2]
                            xo = xoff_y if ring is nc.sync else xoff_s
                            ring.dma_start(
                                tl[:, j * q : (j + 1) * q, :],
                                xt_dyn(w, xo[s], j * q, q),
                            )
                    else:
                        half = NK // 2
                        nc.sync.dma_start(
                            tl[:, 0:half, :], xt_dyn(w, xoff_y[s], 0, half)
                        )
                        nc.scalar.dma_start(
                            tl[:, half:NK, :], xt_dyn(w, xoff_s[s], half, half)
                        )
                    xtj[s] = tl

                hsjr = {0: hsj_d}
                for s, w, _ in tasks[1:]:
                    h16 = work.tile([1, 512], BF16, tag="hsj16", bufs=8)
                    nc.gpsimd.dma_start(h16[:, 0:w], hs_dyn(w, hoff[s]))
                    hr = work.tile([1, 512], F32R, tag="hsj", bufs=8)
                    nc.vector.tensor_copy(hr[:, 0:w], h16[:, 0:w])
                    hsjr[s] = hr

                # ---- bandwidth from the gathered tails (after AG)
                ap_tail = ag_all[0 : cfg.cores * AGT].rearrange(
                    "(r c) -> r c", c=AGT
                )[:, HK : HK + TAIL]
                nc.scalar.dma_start(ag_sb[:], ap_tail)
                psum_s_t = mpsum.tile([NC, 512], F32, tag="R", bufs=2, name="ps1")
                psum_s = psum_s_t[0:1, 0:1]
                nc.tensor.matmul(
                    psum_s,
                    lhsT=bones_col[0 : cfg.cores, :],
                    rhs=ag_sb[:, RPC + D : RPC + D + 1],
                    start=True,
                    stop=True,
                )
                s1 = setup.tile([1, 1], F32, tag="tiny", bufs=8)
                nc.vector.tensor_copy(s1[:], psum_s)
                for ch in range(4):
                    pcg_t = mpsum.tile([128, 512], F32, tag="g", bufs=6, name=f"pcg{ch}")
                    pcg = pcg_t[0:1, 0:512]
                    nc.tensor.matmul(
                        pcg,
                        lhsT=bones_col[0 : cfg.cores, :],
                        rhs=ag_sb[:, RPC + 512 * ch : RPC + 512 * (ch + 1)],
                        start=True,
                        stop=True,
                    )
                    junk_cg = setup.tile([1, 512], BF16, tag="junkcg", bufs=2)
                    nc.scalar.activation(
                        junk_cg[:], pcg, ACTF.Square, accum_out=s2v[:, ch : ch + 1]
                    )
                s2 = setup.tile([1, 1], F32, tag="tiny", bufs=8)
                nc.vector.tensor_reduce(s2[:], s2v[:], axis=AX.X, op=ALU.add)
                t1 = setup.tile([1, 1], F32, tag="tiny", bufs=8)
                t2 = setup.tile([1, 1], F32, tag="tiny", bufs=8)
                bw0 = setup.tile([1, 1], F32, tag="tiny", bufs=8)
                inv0 = setup.tile([1, 1], F32, tag="tiny", bufs=8)
                nc.vector.tensor_scalar_mul(t1[:], s1[:], 1.0 / (N - 1))
                nc.vector.tensor_scalar_mul(t2[:], s2[:], -1.0 / (2.0 * N * (N - 1)))
                nc.vector.tensor_tensor(bw0[:], t1[:], t2[:], op=ALU.add)
                nc.vector.reciprocal(inv0[:], bw0[:])
                sc10 = setup.tile([1, 2 * NL], F32, tag="sc10", bufs=1)
                nc.vector.tensor_scalar_mul(sc10[:], cst_sb[:, 0 : 2 * NL], inv0[:])
                psum_b_t = mpsum.tile([128, 512], F32, tag="g", bufs=6, name="pb")
                psum_b = psum_b_t[:, 0 : 2 * NL]
                nc.tensor.matmul(
                    psum_b, lhsT=ones_row[:], rhs=sc10[:], start=True, stop=True
                )
                nc.vector.tensor_copy(sc[:], psum_b)
                for l in range(NL):
                    nc.vector.tensor_scalar_mul(
                        biases[:, NI * l : NI * (l + 1)],
                        halfsq[:],
                        sc[:, NL + l : NL + l + 1],
                    )

                # ---- main loop: deferred exp/Esum/reduce groups
                pend = []

                def flush_pend():
                    if not pend:
                        return
                    s, w, i, g, psum_R, first, last = pend.pop(0)
                    # levels 4,3,2 straight from ACT; 1,0 by squaring E2 on DVE
                    Es = []
                    for l in (NL - 1, NL - 2, NL - 3):
                        E = work.tile([128, 512], BF16, tag="E", bufs=24)
                        nc.scalar.activation(
                            E[:, 0:w],
                            g[:, 0:w],
                            ACTF.Exp,
                            bias=biases[:, NI * l + i : NI * l + i + 1],
                            scale=sc[:, l : l + 1],
                        )
                        Es.append(E)
                    S = Es[0]
                    for E_next in Es[1:]:
                        S2 = work.tile([128, 512], BF16, tag="E", bufs=24)
                        nc.vector.tensor_tensor(
                            S2[:, 0:w], S[:, 0:w], E_next[:, 0:w], op=ALU.add
                        )
                        S = S2
                    E = Es[-1]
                    for step in range(2):
                        E2 = work.tile([128, 512], BF16, tag="E", bufs=24)
                        nc.vector.tensor_tensor(
                            E2[:, 0:w], E[:, 0:w], E[:, 0:w], op=ALU.mult
                        )
                        S2 = work.tile([128, 512], BF16, tag="E", bufs=24)
                        nc.vector.tensor_tensor(
                            S2[:, 0:w], S[:, 0:w], E2[:, 0:w], op=ALU.add
                        )
                        E = E2
                        S = S2
                    nc.tensor.matmul(
                        psum_R[:, 0:w],
                        lhsT=vown_sb[:, i, :],
                        rhs=S[:, 0:w],
                        start=first,
                        stop=last,
                    )
                    if last:
                        scr = work.tile([NC, 512], F32, tag="scr", bufs=2)
                        nc.vector.tensor_tensor(
                            scr[:, 0:w],
                            psum_R[:, 0:w],
                            vt_sb[:, SLOT_OFF[s] : SLOT_OFF[s] + w],
                            op=ALU.mult,
                        )
                        nc.vector.tensor_reduce(
                            loss_cols[:, s : s + 1], scr[:, 0:w], axis=AX.X, op=ALU.add
                        )

                for s, w, ilist in tasks:
                    psum_R = mpsum.tile([NC, 512], F32, tag="R", bufs=2, name=f"R{s}")
                    for idx, i in enumerate(ilist):
                        if s == 0:
                            g = gdiag[i]  # spilled SBUF copy; no gram here
                        else:
                            g = mpsum.tile(
                                [128, 512], F32, tag="g", bufs=6, name=f"g{s}_{i}"
                            )
                            for k2 in range(NK // 2):
                                nc.tensor.matmul(
                                    g[:, 0:w],
                                    lhsT=xto[
                                        :, 2 * k2 : 2 * k2 + 2, 128 * i : 128 * (i + 1)
                                    ],
                                    rhs=xtj[s][:, 2 * k2 : 2 * k2 + 2, :],
                                    start=(k2 == 0),
                                    stop=False,
                                    perf_mode=mybir.MatmulPerfMode.DoubleRow,
                                )
                            nc.tensor.matmul(
                                g[:, 0:w],
                                lhsT=negs_rowr[:],
                                rhs=hsjr[s][:, 0:w],
                                start=False,
                                stop=True,
                            )
                            if len(pend) > 3:
                                flush_pend()
                            if len(pend) > 4:
                                flush_pend()
                        pend.append(
                            (s, w, i, g, psum_R, idx == 0, idx == len(ilist) - 1)
                        )
                while pend:
                    flush_pend()

                nc.vector.tensor_reduce(
                    lred[:], loss_cols[:, 0:6], axis=AX.X, op=ALU.add
                )
                psum_f = mpsum.tile([1, 1], F32, tag="R", bufs=2)
                nc.tensor.matmul(
                    psum_f[:],
                    lhsT=lred[:],
                    rhs=ones_col[0:NC, :],
                    start=True,
                    stop=True,
                )
                nc.vector.tensor_copy(out_sb[:], psum_f[:])
                nc.sync.dma_start(partial, out_sb[:])

    nc.compile()
    return nc


def host_prep(cfg: Cfg, source, target, s_label, t_label):
    """Slice/encode inputs into per-core in_maps (layout only, no math on X)."""
    X = np.concatenate(
        [np.asarray(source, np.float32), np.asarray(target, np.float32)], 0
    )
    bs = np.asarray(source).shape[0]
    lab = np.concatenate(
        [np.asarray(s_label).astype(np.int64), np.asarray(t_label).astype(np.int64)]
    )
    sign = np.ones(cfg.n, np.float32)
    sign[bs:] = -1.0
    V = np.zeros((cfg.n, cfg.ncls), np.float32)
    V[np.arange(cfg.n), lab] = sign
    Vb = V.astype(ml_dtypes.bfloat16)
    VT = np.ascontiguousarray(V.T)
    VT2 = 2.0 * VT

    NL = cfg.kernel_num
    cst = np.zeros((1, 16), np.float32)
    for l in range(NL):
        cst[0, l] = 2.0 * 2.0 ** (-l)
        cst[0, NL + l] = -2.0 * 2.0 ** (-l)
    cones = np.ones((128, 1), np.float32)
    cbcol = np.ones((128, 1), ml_dtypes.bfloat16)
    crow = np.ones((1, 128), np.float32)
    cnrow = -np.ones((1, 128), np.float32)

    in_maps = []
    for c in range(cfg.cores):
        r0, r1 = c * cfg.rpc, (c + 1) * cfg.rpc
        sub1 = 256 if c >= 4 else 0
        sub2 = 256 - sub1
        r4 = (c + 4) % 8
        slots = [VT[:, r0:r1]]
        for s in (1, 2, 3):
            rs = (c + s) % 8
            slots.append(VT2[:, 512 * rs : 512 * (rs + 1)])
        slots.append(VT2[:, 512 * r4 + sub1 : 512 * r4 + sub1 + 256])
        slots.append(VT2[:, 512 * r4 + sub2 : 512 * r4 + sub2 + 256])
        vt_c = np.concatenate(slots, axis=1).astype(ml_dtypes.bfloat16)
        in_maps.append(
            {
                "xsT": np.ascontiguousarray(X[r0:r1].T),
                "vown": np.ascontiguousarray(Vb[r0:r1]),
                "vt": np.ascontiguousarray(vt_c),
                "cst": cst,
                "cones": cones,
                "cbcol": cbcol,
                "crow": crow,
                "cnrow": cnrow,
            }
        )
    return in_maps


_NC_CACHE = {}


def _get_nc(cfg: Cfg):
    key = cfg
    if key not in _NC_CACHE:
        _NC_CACHE[key] = _build(cfg)
    return _NC_CACHE[key]


def run(inputs: dict, cfg: Cfg = CFG, trace: bool = False):
    from concourse.bass_utils import run_bass_kernel_spmd

    nc = _get_nc(cfg)
    in_maps = host_prep(
        cfg,
        inputs["source"],
        inputs["target"],
        inputs["s_label"],
        inputs["t_label"],
    )
    res = run_bass_kernel_spmd(
        nc, in_maps, core_ids=list(range(cfg.cores)), trace=trace
    )
    bs = np.asarray(inputs["source"]).shape[0]
    total = sum(float(r["partial"][0, 0]) for r in res.results)
    loss = np.float32(total / float(bs) ** 2)
    return np.asarray(loss, dtype=np.float32), res


def kernel(**inputs) -> np.ndarray:
    out, _ = run(inputs)
    return out
